# revision 3
# baseline (speedup 1.0000x reference)
"""Expert-parallel MoE ConditionalFeedForward (SwiGLU) on 8 Trainium2 cores.

Sharding: expert-parallel. Core e holds expert e's weights (w1/w2/w3 slices),
x is replicated. Each core computes its expert's SwiGLU output for all 16
tokens; the host gathers out[t, k] = expert_out[idx[t, k]][t].

Per-core dataflow (all weights pre-transposed + pre-tiled on host, bf16):
  h1[t, o] = sum_d x[t, d] * w1[o, d]     (accumulated over 16 d-tiles in PSUM)
  h3[t, o] = sum_d x[t, d] * w3[o, d]
  g = silu(h1) * h3                        (ACT + DVE, fp32 -> bf16)
  gT via PE transpose (identity matmul)
  out[t, i] += sum_o g[t, o] * w2[i, o]    (accumulated over all o in PSUM)

Weights stream through the PE as the *moving* operand (bf16: 1 col/cycle), so
the tensor engine is never the bottleneck; the kernel is HBM-DMA-bound.
"""
import os
import sys

for _p in ("/opt/trn_rl_repo", "/root/.axon_site", "/root/.axon_site/_ro/trn_rl_repo",
           "/root/.axon_site/_ro/pypackages"):
    if os.path.isdir(_p) and _p not in sys.path:
        sys.path.append(_p)

import numpy as np
import ml_dtypes
from contextlib import ExitStack

from concourse import bacc, mybir, masks
import concourse.bass as bass
import concourse.tile as tile
from concourse.bass_utils import run_bass_kernel_spmd

N_CORES = 8
T = 16          # tokens
D = 2048        # model dim
O = 7168        # intermediate dim
NOC = 14        # o-chunks of 512
OCH = 512
NDT = 16        # d-tiles of 128
NJ = 4          # 128-wide o-subtiles per o-chunk
NIC = 4         # i-chunks of 512 in the output dim
BF16 = mybir.dt.bfloat16
F32 = mybir.dt.float32

_compiled = {}


def _build():
    nc = bacc.Bacc("TRN2", target_bir_lowering=False, debug=False,
                   num_devices=N_CORES)

    xt_d = nc.dram_tensor("xt", [128, NDT, T], BF16, kind="ExternalInput")
    w1_d = nc.dram_tensor("w1", [NOC, NDT, 128, OCH], BF16, kind="ExternalInput")
    w3_d = nc.dram_tensor("w3", [NOC, NDT, 128, OCH], BF16, kind="ExternalInput")
    w2_d = nc.dram_tensor("w2", [NOC, NJ, 128, D], BF16, kind="ExternalInput")
    out_d = nc.dram_tensor("out", [T, D], F32, kind="ExternalOutput")

    with tile.TileContext(nc) as tc, ExitStack() as ctx:
        cpool = ctx.enter_context(tc.tile_pool(name="const", bufs=1))
        w1p = ctx.enter_context(tc.tile_pool(name="w1p", bufs=3))
        w3p = ctx.enter_context(tc.tile_pool(name="w3p", bufs=3))
        w2p = ctx.enter_context(tc.tile_pool(name="w2p", bufs=3))
        sbp = ctx.enter_context(tc.tile_pool(name="sb", bufs=2))
        obp = ctx.enter_context(tc.tile_pool(name="ob", bufs=1))
        ph1 = ctx.enter_context(tc.tile_pool(name="ph1", bufs=1, space="PSUM"))
        ph3 = ctx.enter_context(tc.tile_pool(name="ph3", bufs=1, space="PSUM"))
        pgt = ctx.enter_context(tc.tile_pool(name="pgt", bufs=1, space="PSUM"))
        pou = ctx.enter_context(tc.tile_pool(name="pou", bufs=1, space="PSUM"))

        xt = cpool.tile([128, NDT, T], BF16)
        nc.sync.dma_start(xt[:], xt_d.ap()[:])
        ident = cpool.tile([T, T], BF16)
        masks.make_identity(nc, ident[:])

        pouts = [pou.tile([T, OCH], F32, name=f"po{i}", tag=f"po{i}")
                 for i in range(NIC)]

        for oc in range(NOC):
            w1s = w1p.tile([128, NDT, OCH], BF16)
            nc.sync.dma_start(w1s[:], w1_d.ap()[oc].rearrange("a p b -> p a b"))
            w3s = w3p.tile([128, NDT, OCH], BF16)
            nc.sync.dma_start(w3s[:], w3_d.ap()[oc].rearrange("a p b -> p a b"))
            w2s = w2p.tile([128, NJ, D], BF16)
            nc.sync.dma_start(w2s[:], w2_d.ap()[oc].rearrange("a p b -> p a b"))

            h1 = ph1.tile([T, OCH], F32)
            h3 = ph3.tile([T, OCH], F32)
            for dt in range(NDT):
                nc.tensor.matmul(h1[:], xt[:, dt, :], w1s[:, dt, :],
                                 start=(dt == 0), stop=(dt == NDT - 1))
            for dt in range(NDT):
                nc.tensor.matmul(h3[:], xt[:, dt, :], w3s[:, dt, :],
                                 start=(dt == 0), stop=(dt == NDT - 1))

            sil = sbp.tile([T, OCH], F32, tag="sil")
            nc.scalar.activation(sil[:], h1[:],
                                 mybir.ActivationFunctionType.Silu)
            g = sbp.tile([T, OCH], BF16, tag="g")
            nc.vector.tensor_mul(g[:], sil[:], h3[:])

            gt_ps = pgt.tile([128, NJ, T], BF16)
            for j in range(NJ):
                nc.tensor.matmul(gt_ps[:, j, :], g[:, j * 128:(j + 1) * 128],
                                 ident[:], is_transpose=True)
            gt = sbp.tile([128, NJ, T], BF16, tag="gt")
            nc.vector.tensor_copy(gt[:], gt_ps[:])

            for j in range(NJ):
                for ic in range(NIC):
                    nc.tensor.matmul(
                        pouts[ic][:], gt[:, j, :],
                        w2s[:, j, ic * OCH:(ic + 1) * OCH],
                        start=(oc == 0 and j == 0),
                        stop=(oc == NOC - 1 and j == NJ - 1))

        outs = obp.tile([T, D], F32)
        for ic in range(NIC):
            nc.vector.tensor_copy(outs[:, ic * OCH:(ic + 1) * OCH], pouts[ic][:])
        nc.sync.dma_start(out_d.ap()[:], outs[:])

    nc.compile()
    return nc


def get_nc():
    if "nc" not in _compiled:
        _compiled["nc"] = _build()
    return _compiled["nc"]


def prep_inputs(x, w1, w2, w3):
    """Host-side shard + retile + bf16 cast. Returns per-core input maps."""
    bf = ml_dtypes.bfloat16
    # xt[p, dt, t] = x[t, dt*128 + p]
    xt = np.ascontiguousarray(
        x.T.reshape(NDT, 128, T).transpose(1, 0, 2)).astype(bf)
    in_maps = []
    for e in range(N_CORES):
        # w1t[d, o]; tile [oc, dt, p, o] = w1t[dt*128+p, oc*512+o]
        w1b = np.ascontiguousarray(
            w1[e].T.reshape(NDT, 128, NOC, OCH).transpose(2, 0, 1, 3)).astype(bf)
        w3b = np.ascontiguousarray(
            w3[e].T.reshape(NDT, 128, NOC, OCH).transpose(2, 0, 1, 3)).astype(bf)
        # w2t[o, i]; tile [oc, j, p, i] = w2t[oc*512 + j*128 + p, i]
        w2b = np.ascontiguousarray(w2[e].T.reshape(NOC, NJ, 128, D)).astype(bf)
        in_maps.append({"xt": xt, "w1": w1b, "w3": w3b, "w2": w2b})
    return in_maps


def kernel(x, expert_indices, w1, w2, w3):
    x = np.asarray(x, dtype=np.float32)
    expert_indices = np.asarray(expert_indices)
    nc = get_nc()
    in_maps = prep_inputs(np.asarray(x), np.asarray(w1), np.asarray(w2),
                          np.asarray(w3))
    res = run_bass_kernel_spmd(nc, in_maps, list(range(N_CORES)))
    expert_outs = np.stack([res.results[e]["out"] for e in range(N_CORES)])
    out = expert_outs[expert_indices, np.arange(T)[:, None]]
    return out.astype(np.float32)


# revision 6
# speedup vs baseline: 655.7980x; 655.7980x over previous
"""Expert-parallel MoE ConditionalFeedForward (SwiGLU) on 8 Trainium2 cores.

Sharding: expert-parallel. Core e holds expert e's weights (w1/w2/w3 slices),
x is replicated. Each core computes its expert's SwiGLU output for all 16
tokens; the host gathers out[t, k] = expert_out[idx[t, k]][t].

Per-core dataflow (all weights pre-transposed + pre-tiled on host, bf16):
  h1[t, o] = sum_d x[t, d] * w1[o, d]     (accumulated over 16 d-tiles in PSUM)
  h3[t, o] = sum_d x[t, d] * w3[o, d]
  g = silu(h1) * h3                        (ACT + DVE, fp32 -> bf16)
  gT via PE transpose (identity matmul)
  out[t, i] += sum_o g[t, o] * w2[i, o]    (accumulated over all o in PSUM)

Weights stream through the PE as the *moving* operand (bf16: 1 col/cycle), so
the tensor engine is never the bottleneck; the kernel is HBM-DMA-bound.
"""
import os
import sys

for _p in ("/opt/trn_rl_repo", "/root/.axon_site", "/root/.axon_site/_ro/trn_rl_repo",
           "/root/.axon_site/_ro/pypackages"):
    if os.path.isdir(_p) and _p not in sys.path:
        sys.path.append(_p)

import numpy as np
import ml_dtypes
from contextlib import ExitStack

from concourse import bacc, mybir, masks
import concourse.bass as bass
import concourse.tile as tile
from concourse.bass_utils import run_bass_kernel_spmd

N_CORES = 8
T = 16          # tokens
D = 2048        # model dim
O = 7168        # intermediate dim
NOC = 14        # o-chunks of 512
OCH = 512
NDT = 16        # d-tiles of 128
NJ = 4          # 128-wide o-subtiles per o-chunk
NIC = 4         # i-chunks of 512 in the output dim
BF16 = mybir.dt.bfloat16
F32 = mybir.dt.float32

_compiled = {}


def _build(reps=1):
    nc = bacc.Bacc("TRN2", target_bir_lowering=False, debug=False,
                   num_devices=N_CORES)

    xt_d = nc.dram_tensor("xt", [128, NDT, T], BF16, kind="ExternalInput")
    w1_d = nc.dram_tensor("w1", [NOC, NDT, 128, OCH], BF16, kind="ExternalInput")
    w3_d = nc.dram_tensor("w3", [NOC, NDT, 128, OCH], BF16, kind="ExternalInput")
    w2_d = nc.dram_tensor("w2", [NOC, NJ, 128, D], BF16, kind="ExternalInput")
    out_d = nc.dram_tensor("out", [T, D], F32, kind="ExternalOutput")

    with tile.TileContext(nc) as tc, ExitStack() as ctx:
        cpool = ctx.enter_context(tc.tile_pool(name="const", bufs=1))
        w1p = ctx.enter_context(tc.tile_pool(name="w1p", bufs=3))
        w3p = ctx.enter_context(tc.tile_pool(name="w3p", bufs=3))
        w2p = ctx.enter_context(tc.tile_pool(name="w2p", bufs=3))
        sbp = ctx.enter_context(tc.tile_pool(name="sb", bufs=2))
        obp = ctx.enter_context(tc.tile_pool(name="ob", bufs=1))
        ph1 = ctx.enter_context(tc.tile_pool(name="ph1", bufs=1, space="PSUM"))
        ph3 = ctx.enter_context(tc.tile_pool(name="ph3", bufs=1, space="PSUM"))
        pgt = ctx.enter_context(tc.tile_pool(name="pgt", bufs=1, space="PSUM"))
        pou = ctx.enter_context(tc.tile_pool(name="pou", bufs=1, space="PSUM"))

        xt = cpool.tile([128, NDT, T], BF16)
        nc.sync.dma_start(xt[:], xt_d.ap()[:])
        ident = cpool.tile([T, T], BF16)
        masks.make_identity(nc, ident[:])

        pouts = [pou.tile([T, OCH], F32, name=f"po{i}", tag=f"po{i}")
                 for i in range(NIC)]

        for rep in range(reps):
            _emit_body(nc, tc, xt, ident, pouts, w1p, w3p, w2p, sbp, pgt,
                       ph1, ph3, w1_d, w3_d, w2_d)

        outs = obp.tile([T, D], F32)
        for ic in range(NIC):
            nc.vector.tensor_copy(outs[:, ic * OCH:(ic + 1) * OCH], pouts[ic][:])
        nc.sync.dma_start(out_d.ap()[:], outs[:])

    nc.compile()
    return nc


def _emit_body(nc, tc, xt, ident, pouts, w1p, w3p, w2p, sbp, pgt, ph1, ph3,
               w1_d, w3_d, w2_d):
    if True:
        for oc in range(NOC):
            w1s = w1p.tile([128, NDT, OCH], BF16)
            nc.sync.dma_start(w1s[:], w1_d.ap()[oc].rearrange("a p b -> p a b"))
            w3s = w3p.tile([128, NDT, OCH], BF16)
            nc.sync.dma_start(w3s[:], w3_d.ap()[oc].rearrange("a p b -> p a b"))
            w2s = w2p.tile([128, NJ, D], BF16)
            nc.sync.dma_start(w2s[:], w2_d.ap()[oc].rearrange("a p b -> p a b"))

            h1 = ph1.tile([T, OCH], F32)
            h3 = ph3.tile([T, OCH], F32)
            for dt in range(NDT):
                nc.tensor.matmul(h1[:], xt[:, dt, :], w1s[:, dt, :],
                                 start=(dt == 0), stop=(dt == NDT - 1))
            for dt in range(NDT):
                nc.tensor.matmul(h3[:], xt[:, dt, :], w3s[:, dt, :],
                                 start=(dt == 0), stop=(dt == NDT - 1))

            sil = sbp.tile([T, OCH], F32, tag="sil")
            nc.scalar.activation(sil[:], h1[:],
                                 mybir.ActivationFunctionType.Silu)
            g = sbp.tile([T, OCH], BF16, tag="g")
            nc.vector.tensor_mul(g[:], sil[:], h3[:])

            gt_ps = pgt.tile([128, NJ, T], BF16)
            for j in range(NJ):
                nc.tensor.matmul(gt_ps[:, j, :], g[:, j * 128:(j + 1) * 128],
                                 ident[:], is_transpose=True)
            gt = sbp.tile([128, NJ, T], BF16, tag="gt")
            nc.vector.tensor_copy(gt[:], gt_ps[:])

            for j in range(NJ):
                for ic in range(NIC):
                    nc.tensor.matmul(
                        pouts[ic][:], gt[:, j, :],
                        w2s[:, j, ic * OCH:(ic + 1) * OCH],
                        start=(oc == 0 and j == 0),
                        stop=(oc == NOC - 1 and j == NJ - 1))


def get_nc(reps=1):
    if reps not in _compiled:
        _compiled[reps] = _build(reps)
    return _compiled[reps]


def prep_inputs(x, w1, w2, w3):
    """Host-side shard + retile + bf16 cast. Returns per-core input maps."""
    bf = ml_dtypes.bfloat16
    # xt[p, dt, t] = x[t, dt*128 + p]
    xt = np.ascontiguousarray(
        x.T.reshape(NDT, 128, T).transpose(1, 0, 2)).astype(bf)
    in_maps = []
    for e in range(N_CORES):
        # w1t[d, o]; tile [oc, dt, p, o] = w1t[dt*128+p, oc*512+o]
        w1b = np.ascontiguousarray(
            w1[e].T.reshape(NDT, 128, NOC, OCH).transpose(2, 0, 1, 3)).astype(bf)
        w3b = np.ascontiguousarray(
            w3[e].T.reshape(NDT, 128, NOC, OCH).transpose(2, 0, 1, 3)).astype(bf)
        # w2t[o, i]; tile [oc, j, p, i] = w2t[oc*512 + j*128 + p, i]
        w2b = np.ascontiguousarray(w2[e].T.reshape(NOC, NJ, 128, D)).astype(bf)
        in_maps.append({"xt": xt, "w1": w1b, "w3": w3b, "w2": w2b})
    return in_maps


def kernel(x, expert_indices, w1, w2, w3):
    x = np.asarray(x, dtype=np.float32)
    expert_indices = np.asarray(expert_indices)
    nc = get_nc()
    in_maps = prep_inputs(np.asarray(x), np.asarray(w1), np.asarray(w2),
                          np.asarray(w3))
    res = run_bass_kernel_spmd(nc, in_maps, list(range(N_CORES)))
    expert_outs = np.stack([res.results[e]["out"] for e in range(N_CORES)])
    out = expert_outs[expert_indices, np.arange(T)[:, None]]
    return out.astype(np.float32)


# revision 7
# speedup vs baseline: 725.4401x; 1.1062x over previous
"""Expert-parallel MoE ConditionalFeedForward (SwiGLU) on 8 Trainium2 cores.

Sharding: expert-parallel. Core e holds expert e's weights (w1/w2/w3 slices),
x is replicated. Each core computes its expert's SwiGLU output for all 16
tokens; the host gathers out[t, k] = expert_out[idx[t, k]][t].

Per-core dataflow (all weights pre-transposed + pre-tiled on host, bf16):
  h1[t, o] = sum_d x[t, d] * w1[o, d]     (accumulated over 16 d-tiles in PSUM)
  h3[t, o] = sum_d x[t, d] * w3[o, d]
  g = silu(h1) * h3                        (ACT + DVE, fp32 -> bf16)
  gT via PE transpose (identity matmul)
  out[t, i] += sum_o g[t, o] * w2[i, o]    (accumulated over all o in PSUM)

Weights stream through the PE as the *moving* operand (bf16: 1 col/cycle), so
the tensor engine is never the bottleneck; the kernel is HBM-DMA-bound.
"""
import os
import sys

for _p in ("/opt/trn_rl_repo", "/root/.axon_site", "/root/.axon_site/_ro/trn_rl_repo",
           "/root/.axon_site/_ro/pypackages"):
    if os.path.isdir(_p) and _p not in sys.path:
        sys.path.append(_p)

import numpy as np
import ml_dtypes
from contextlib import ExitStack

from concourse import bacc, mybir, masks
import concourse.bass as bass
import concourse.tile as tile
from concourse.bass_utils import run_bass_kernel_spmd

N_CORES = 8
T = 16          # tokens
D = 2048        # model dim
O = 7168        # intermediate dim
NDT = 16        # d-tiles of 128
NIC = 4         # i-chunks of 512 in the output dim
OCH = 256       # o-chunk width
NOC = O // OCH  # number of o-chunks
NJ = OCH // 128  # 128-wide o-subtiles per o-chunk
WBUFS = 3       # weight-slab double/triple buffering
BF16 = mybir.dt.bfloat16
F32 = mybir.dt.float32

_compiled = {}


def _build(reps=1):
    nc = bacc.Bacc("TRN2", target_bir_lowering=False, debug=False,
                   num_devices=N_CORES)

    xt_d = nc.dram_tensor("xt", [128, NDT, T], BF16, kind="ExternalInput")
    w1_d = nc.dram_tensor("w1", [NOC, NDT, 128, OCH], BF16, kind="ExternalInput")
    w3_d = nc.dram_tensor("w3", [NOC, NDT, 128, OCH], BF16, kind="ExternalInput")
    w2_d = nc.dram_tensor("w2", [NOC, NJ, 128, D], BF16, kind="ExternalInput")
    out_d = nc.dram_tensor("out", [T, D], F32, kind="ExternalOutput")

    with tile.TileContext(nc) as tc, ExitStack() as ctx:
        cpool = ctx.enter_context(tc.tile_pool(name="const", bufs=1))
        w1p = ctx.enter_context(tc.tile_pool(name="w1p", bufs=WBUFS))
        w3p = ctx.enter_context(tc.tile_pool(name="w3p", bufs=WBUFS))
        w2p = ctx.enter_context(tc.tile_pool(name="w2p", bufs=WBUFS))
        sbp = ctx.enter_context(tc.tile_pool(name="sb", bufs=2))
        obp = ctx.enter_context(tc.tile_pool(name="ob", bufs=1))
        ph1 = ctx.enter_context(tc.tile_pool(name="ph1", bufs=1, space="PSUM"))
        ph3 = ctx.enter_context(tc.tile_pool(name="ph3", bufs=1, space="PSUM"))
        pgt = ctx.enter_context(tc.tile_pool(name="pgt", bufs=1, space="PSUM"))
        pou = ctx.enter_context(tc.tile_pool(name="pou", bufs=1, space="PSUM"))

        xt = cpool.tile([128, NDT, T], BF16)
        nc.sync.dma_start(xt[:], xt_d.ap()[:])
        ident = cpool.tile([T, T], BF16)
        masks.make_identity(nc, ident[:])

        pouts = [pou.tile([T, 512], F32, name=f"po{i}", tag=f"po{i}")
                 for i in range(NIC)]

        for rep in range(reps):
            for oc in range(NOC):
                w1s = w1p.tile([128, NDT, OCH], BF16)
                nc.sync.dma_start(w1s[:], w1_d.ap()[oc].rearrange("a p b -> p a b"))
                w3s = w3p.tile([128, NDT, OCH], BF16)
                nc.sync.dma_start(w3s[:], w3_d.ap()[oc].rearrange("a p b -> p a b"))
                w2s = w2p.tile([128, NJ, D], BF16)
                nc.sync.dma_start(w2s[:], w2_d.ap()[oc].rearrange("a p b -> p a b"))

                h1 = ph1.tile([T, OCH], F32)
                h3 = ph3.tile([T, OCH], F32)
                for dt in range(NDT):
                    nc.tensor.matmul(h1[:], xt[:, dt, :], w1s[:, dt, :],
                                     start=(dt == 0), stop=(dt == NDT - 1))
                for dt in range(NDT):
                    nc.tensor.matmul(h3[:], xt[:, dt, :], w3s[:, dt, :],
                                     start=(dt == 0), stop=(dt == NDT - 1))

                sil = sbp.tile([T, OCH], F32, tag="sil")
                nc.scalar.activation(sil[:], h1[:],
                                     mybir.ActivationFunctionType.Silu)
                g = sbp.tile([T, OCH], BF16, tag="g")
                nc.vector.tensor_mul(g[:], sil[:], h3[:])

                gt_ps = pgt.tile([128, NJ, T], BF16)
                for j in range(NJ):
                    nc.tensor.matmul(gt_ps[:, j, :], g[:, j * 128:(j + 1) * 128],
                                     ident[:], is_transpose=True)
                gt = sbp.tile([128, NJ, T], BF16, tag="gt")
                nc.vector.tensor_copy(gt[:], gt_ps[:])

                for j in range(NJ):
                    for ic in range(NIC):
                        nc.tensor.matmul(
                            pouts[ic][:], gt[:, j, :],
                            w2s[:, j, ic * 512:(ic + 1) * 512],
                            start=(oc == 0 and j == 0),
                            stop=(oc == NOC - 1 and j == NJ - 1))

        outs = obp.tile([T, D], F32)
        for ic in range(NIC):
            nc.vector.tensor_copy(outs[:, ic * 512:(ic + 1) * 512], pouts[ic][:])
        nc.sync.dma_start(out_d.ap()[:], outs[:])

    nc.compile()
    return nc


def get_nc(reps=1):
    if reps not in _compiled:
        _compiled[reps] = _build(reps)
    return _compiled[reps]


def prep_inputs(x, w1, w2, w3):
    """Host-side shard + retile + bf16 cast. Returns per-core input maps."""
    bf = ml_dtypes.bfloat16
    # xt[p, dt, t] = x[t, dt*128 + p]
    xt = np.ascontiguousarray(
        x.T.reshape(NDT, 128, T).transpose(1, 0, 2)).astype(bf)
    in_maps = []
    for e in range(N_CORES):
        # w1t[d, o]; tile [oc, dt, p, o] = w1t[dt*128+p, oc*OCH+o]
        w1b = np.ascontiguousarray(
            w1[e].T.reshape(NDT, 128, NOC, OCH).transpose(2, 0, 1, 3)).astype(bf)
        w3b = np.ascontiguousarray(
            w3[e].T.reshape(NDT, 128, NOC, OCH).transpose(2, 0, 1, 3)).astype(bf)
        # w2t[o, i]; tile [oc, j, p, i] = w2t[oc*OCH + j*128 + p, i]
        w2b = np.ascontiguousarray(w2[e].T.reshape(NOC, NJ, 128, D)).astype(bf)
        in_maps.append({"xt": xt, "w1": w1b, "w3": w3b, "w2": w2b})
    return in_maps


def kernel(x, expert_indices, w1, w2, w3):
    x = np.asarray(x, dtype=np.float32)
    expert_indices = np.asarray(expert_indices)
    nc = get_nc()
    in_maps = prep_inputs(np.asarray(x), np.asarray(w1), np.asarray(w2),
                          np.asarray(w3))
    res = run_bass_kernel_spmd(nc, in_maps, list(range(N_CORES)))
    expert_outs = np.stack([res.results[e]["out"] for e in range(N_CORES)])
    out = expert_outs[expert_indices, np.arange(T)[:, None]]
    return out.astype(np.float32)


# revision 10
# speedup vs baseline: 1345.9780x; 1.8554x over previous
"""Expert-parallel MoE ConditionalFeedForward (SwiGLU) on 8 Trainium2 cores.

Sharding: expert-parallel. Core e holds expert e's weights (w1/w2/w3 slices),
x is replicated. Each core computes its expert's SwiGLU output for all 16
tokens; the host gathers out[t, k] = expert_out[idx[t, k]][t].

Per-core dataflow (all weights pre-transposed + pre-tiled on host, bf16):
  h1[t, o] = sum_d x[t, d] * w1[o, d]     (accumulated over 16 d-tiles in PSUM)
  h3[t, o] = sum_d x[t, d] * w3[o, d]
  g = silu(h1) * h3                        (ACT + DVE, fp32 -> bf16)
  gT via PE transpose (identity matmul)
  out[t, i] += sum_o g[t, o] * w2[i, o]    (accumulated over all o in PSUM)

Weights stream through the PE as the *moving* operand (bf16: 1 col/cycle), so
the tensor engine is never the bottleneck; the kernel is HBM-DMA-bound.
"""
import os
import sys

for _p in ("/opt/trn_rl_repo", "/root/.axon_site", "/root/.axon_site/_ro/trn_rl_repo",
           "/root/.axon_site/_ro/pypackages"):
    if os.path.isdir(_p) and _p not in sys.path:
        sys.path.append(_p)

import numpy as np
import ml_dtypes
from contextlib import ExitStack

from concourse import bacc, mybir, masks
import concourse.bass as bass
import concourse.tile as tile
from concourse.bass_utils import run_bass_kernel_spmd

N_CORES = 8
T = 16          # tokens
D = 2048        # model dim
O = 7168        # intermediate dim
NDT = 16        # d-tiles of 128
NIC = 4         # i-chunks of 512 in the output dim
OCH = 256       # o-chunk width
NOC = O // OCH  # number of o-chunks
NJ = OCH // 128  # 128-wide o-subtiles per o-chunk
WBUFS = 4       # weight-slab double/triple buffering
BF16 = mybir.dt.bfloat16
F32 = mybir.dt.float32

_compiled = {}


def _build(reps=1):
    nc = bacc.Bacc("TRN2", target_bir_lowering=False, debug=False,
                   num_devices=N_CORES)

    xt_d = nc.dram_tensor("xt", [128, NDT, T], BF16, kind="ExternalInput")
    w1_d = nc.dram_tensor("w1", [NOC, NDT, 128, OCH], BF16, kind="ExternalInput")
    w3_d = nc.dram_tensor("w3", [NOC, NDT, 128, OCH], BF16, kind="ExternalInput")
    w2_d = nc.dram_tensor("w2", [NOC, NJ, 128, D], BF16, kind="ExternalInput")
    out_d = nc.dram_tensor("out", [T, D], F32, kind="ExternalOutput")

    with tile.TileContext(nc) as tc, ExitStack() as ctx:
        cpool = ctx.enter_context(tc.tile_pool(name="const", bufs=1))
        w1p = ctx.enter_context(tc.tile_pool(name="w1p", bufs=WBUFS))
        w3p = ctx.enter_context(tc.tile_pool(name="w3p", bufs=WBUFS))
        w2p = ctx.enter_context(tc.tile_pool(name="w2p", bufs=WBUFS))
        sbp = ctx.enter_context(tc.tile_pool(name="sb", bufs=3))
        obp = ctx.enter_context(tc.tile_pool(name="ob", bufs=1))
        php = ctx.enter_context(tc.tile_pool(name="php", bufs=2, space="PSUM"))
        pgt = ctx.enter_context(tc.tile_pool(name="pgt", bufs=1, space="PSUM"))
        pou = ctx.enter_context(tc.tile_pool(name="pou", bufs=1, space="PSUM"))

        xt = cpool.tile([128, NDT, T], BF16)
        nc.sync.dma_start(xt[:], xt_d.ap()[:])
        ident = cpool.tile([T, T], BF16)
        masks.make_identity(nc, ident[:])

        pouts = [pou.tile([T, 512], F32, name=f"po{i}", tag=f"po{i}")
                 for i in range(NIC)]

        for rep in range(reps):
            for oc in range(NOC):
                w1s = w1p.tile([128, NDT, OCH], BF16)
                nc.sync.dma_start(w1s[:], w1_d.ap()[oc].rearrange("a p b -> p a b"))
                w3s = w3p.tile([128, NDT, OCH], BF16)
                nc.sync.dma_start(w3s[:], w3_d.ap()[oc].rearrange("a p b -> p a b"))
                w2s = w2p.tile([128, NJ, D], BF16)
                nc.sync.dma_start(w2s[:], w2_d.ap()[oc].rearrange("a p b -> p a b"))

                h = php.tile([T, 2, OCH], F32)
                h1 = h[:, 0, :]
                h3 = h[:, 1, :]
                for dt in range(NDT):
                    nc.tensor.matmul(h1[:], xt[:, dt, :], w1s[:, dt, :],
                                     start=(dt == 0), stop=(dt == NDT - 1))
                for dt in range(NDT):
                    nc.tensor.matmul(h3[:], xt[:, dt, :], w3s[:, dt, :],
                                     start=(dt == 0), stop=(dt == NDT - 1))

                sil = sbp.tile([T, OCH], F32, tag="sil")
                nc.scalar.activation(sil[:], h1[:],
                                     mybir.ActivationFunctionType.Silu)
                g = sbp.tile([T, OCH], BF16, tag="g")
                nc.vector.tensor_mul(g[:], sil[:], h3[:])

                gt_ps = pgt.tile([128, NJ, T], BF16)
                for j in range(NJ):
                    nc.tensor.matmul(gt_ps[:, j, :], g[:, j * 128:(j + 1) * 128],
                                     ident[:], is_transpose=True)
                gt = sbp.tile([128, NJ, T], BF16, tag="gt")
                nc.vector.tensor_copy(gt[:], gt_ps[:])

                for j in range(NJ):
                    for ic in range(NIC):
                        nc.tensor.matmul(
                            pouts[ic][:], gt[:, j, :],
                            w2s[:, j, ic * 512:(ic + 1) * 512],
                            start=(oc == 0 and j == 0),
                            stop=(oc == NOC - 1 and j == NJ - 1))

        outs = obp.tile([T, D], F32)
        for ic in range(NIC):
            nc.vector.tensor_copy(outs[:, ic * 512:(ic + 1) * 512], pouts[ic][:])
        nc.sync.dma_start(out_d.ap()[:], outs[:])

    nc.compile()
    return nc


def get_nc(reps=1):
    if reps not in _compiled:
        _compiled[reps] = _build(reps)
    return _compiled[reps]


def prep_inputs(x, w1, w2, w3):
    """Host-side shard + retile + bf16 cast. Returns per-core input maps."""
    bf = ml_dtypes.bfloat16
    # xt[p, dt, t] = x[t, dt*128 + p]
    xt = np.ascontiguousarray(
        x.T.reshape(NDT, 128, T).transpose(1, 0, 2)).astype(bf)
    in_maps = []
    for e in range(N_CORES):
        # w1t[d, o]; tile [oc, dt, p, o] = w1t[dt*128+p, oc*OCH+o]
        w1b = np.ascontiguousarray(
            w1[e].T.reshape(NDT, 128, NOC, OCH).transpose(2, 0, 1, 3)).astype(bf)
        w3b = np.ascontiguousarray(
            w3[e].T.reshape(NDT, 128, NOC, OCH).transpose(2, 0, 1, 3)).astype(bf)
        # w2t[o, i]; tile [oc, j, p, i] = w2t[oc*OCH + j*128 + p, i]
        w2b = np.ascontiguousarray(w2[e].T.reshape(NOC, NJ, 128, D)).astype(bf)
        in_maps.append({"xt": xt, "w1": w1b, "w3": w3b, "w2": w2b})
    return in_maps


def kernel(x, expert_indices, w1, w2, w3):
    x = np.asarray(x, dtype=np.float32)
    expert_indices = np.asarray(expert_indices)
    nc = get_nc()
    in_maps = prep_inputs(np.asarray(x), np.asarray(w1), np.asarray(w2),
                          np.asarray(w3))
    res = run_bass_kernel_spmd(nc, in_maps, list(range(N_CORES)))
    expert_outs = np.stack([res.results[e]["out"] for e in range(N_CORES)])
    out = expert_outs[expert_indices, np.arange(T)[:, None]]
    return out.astype(np.float32)
